# revision 11
# baseline (speedup 1.0000x reference)
"""BEV detection loss on 8 Trainium2 NeuronCores.

Strategy (data-parallel over batch, one batch element per core):
  - The loss touches cls_logits / box_preds only at positive cells (<= 64
    per batch element, from the first-come-wins scatter assignment).  The
    host does the tiny assignment + per-positive-cell math (CE, smooth-L1,
    softplus at positive cells) in float32; only the full scan
    sum(softplus(obj_logits)) over 262144 cells per core runs on device.
  - Device algorithm per core, on a [128, 2048] layout:
      x (fp8 e4m3, quantized on host -- 2e-2 rel-err budget, measured
        impact 2e-4) --DMA--> SBUF in 4 chunks on the SP HWDGE queue
      ACT: E = exp(x)            (bf16)
      DVE: F = 1 + E             (bf16, tensor-scalar 4x mode)
           three contiguous-halving products -> P = prod of 8 F's
           (group order is irrelevant for the sum; contiguous halves
           keep the DVE 2x packed mode)
      ACT: ln(P) over 256 cols with accum_out -> per-partition sums
    sum softplus = sum ln(1+e^x) = sum ln(P).  Products of 8 factors
    stay < e^44, well inside bf16 range.
  - Only the SP dynamic-HWDGE DMA queue is declared (dropping the unused
    Pool/Activation queues shortens the NEFF prelude/drain measurably).
  - Host combines per-core partial sums with the positives terms into the
    globally-consistent pos_weight and means (float32 throughout).
"""

import sys

import ml_dtypes
import numpy as np

sys.path.insert(0, "/opt/trn_rl_repo")

import concourse.bacc as bacc  # noqa: E402
import concourse.mybir as mybir  # noqa: E402
from concourse.bass_utils import run_bass_kernel_spmd  # noqa: E402

# BEV grid constants (must match the reference)
X_MIN = np.float32(-51.2)
X_MAX = np.float32(51.2)
Y_MIN = np.float32(-51.2)
Y_MAX = np.float32(51.2)
RES = np.float32(0.2)
BEV_W = 512
BEV_H = 512
NUM_CELLS = BEV_W * BEV_H  # 262144
CLS_WEIGHT = np.float32(1.0)
BOX_WEIGHT = np.float32(1.0)

N_CORES = 8
P_DIM = 128
COLS = NUM_CELLS // P_DIM  # 2048
C = 10
D = 7

import os

if os.environ.get("BEV_IN_DT", "fp8") == "bf16":
    IN_DT = mybir.dt.bfloat16
    IN_NP_DT = ml_dtypes.bfloat16
else:
    IN_DT = mybir.dt.float8e4
    IN_NP_DT = ml_dtypes.float8_e4m3
# chunk boundaries: small first chunk so ACT starts early; small last chunk
# (handled without DVE folding) so the post-last-arrival tail is short
FOLD_CHUNKS = [(0, 384), (384, 1088), (1088, 1792)]  # folded 8:1 via DVE products
TAIL_LO, TAIL_HI = 1792, 2048  # pure-ACT ln(1+e^x) chunk
CHUNKS = FOLD_CHUNKS + [(TAIL_LO, TAIL_HI)]
P3_COLS = sum((hi - lo) // 8 for lo, hi in FOLD_CHUNKS)  # 224
OUT_W = 6  # >=24B DMA descriptors (4B-descriptor DMAs complete slowly)
WAIT_OUT = os.environ.get("BEV_WAIT_OUT", "1") == "1"


class _Bacc(bacc.Bacc):
    """Bacc whose constructor-time all_engine_barrier is skipped.

    The barrier only guards the const-pool memsets against engine use; this
    kernel first reads a const (the Ln bias) several microseconds after the
    memsets complete, so the barrier only delays the first input DMA."""

    def all_engine_barrier(self, **kw):
        return None

_CACHE = {}


def _build_program():
    f32 = mybir.dt.float32
    bf16 = mybir.dt.bfloat16
    AF = mybir.ActivationFunctionType

    nc = _Bacc("TRN2", debug=False, target_bir_lowering=False, num_devices=N_CORES)
    # Only the SP HWDGE dynamic queue is used; dropping the Pool/Act queue
    # declarations shortens NRT queue setup and the end-of-NEFF drain.
    nc.m.queues = [q for q in nc.m.queues if q.name == "qSPDynamicHW"]

    in_obj = nc.dram_tensor("in_obj", [P_DIM, COLS], IN_DT, kind="ExternalInput").ap()
    out_all = nc.dram_tensor("out_all", [P_DIM, OUT_W], f32, kind="ExternalOutput").ap()

    # Raw bass (no TileContext): dependencies are hand-managed semaphores.
    # Intra-engine ordering is program order — no semaphore chaining between
    # same-engine ops, which removes the per-op sync overhead and the tile
    # exit-block drains/barriers.
    x = nc.alloc_sbuf_tensor("x", [P_DIM, COLS], IN_DT)
    E = nc.alloc_sbuf_tensor("E", [P_DIM, COLS], bf16)
    F = nc.alloc_sbuf_tensor("F", [P_DIM, FOLD_CHUNKS[-1][1]], bf16)
    P1b = nc.alloc_sbuf_tensor("P1b", [P_DIM, FOLD_CHUNKS[-1][1] // 2], bf16)
    P2b = nc.alloc_sbuf_tensor("P2b", [P_DIM, FOLD_CHUNKS[-1][1] // 4], bf16)
    P3 = nc.alloc_sbuf_tensor("P3", [P_DIM, P3_COLS], bf16)
    Lt = nc.alloc_sbuf_tensor("Lt", [P_DIM, TAIL_HI - TAIL_LO], f32)
    Lg = nc.alloc_sbuf_tensor("Lg", [P_DIM, P3_COLS], f32)
    out = nc.alloc_sbuf_tensor("out", [P_DIM, OUT_W], f32)

    s_in = [nc.alloc_semaphore(f"s_in{c}") for c in range(len(CHUNKS))]
    s_exp = nc.alloc_semaphore("s_exp")
    s_dve = nc.alloc_semaphore("s_dve")
    s_act = nc.alloc_semaphore("s_act")
    s_out = nc.alloc_semaphore("s_out")

    # SP: input chunk DMAs, then the output DMA once ACT is done
    for c, (lo, hi) in enumerate(CHUNKS):
        nc.sync.dma_start(out=x[:, lo:hi], in_=in_obj[:, lo:hi]).then_inc(s_in[c], 16)

    # ACT: warmup (data-independent, pulls the exp/ln table load to block
    # start where it overlaps the input DMA), per-chunk exp, tail ln(1+e),
    # global ln over folded products
    nc.scalar.activation(Lg[:, 0:1], Lg[:, 0:1], AF.Exp, scale=0.0)
    for c, (lo, hi) in enumerate(FOLD_CHUNKS):
        nc.scalar.wait_ge(s_in[c], 16)
        nc.scalar.activation(E[:, lo:hi], x[:, lo:hi], AF.Exp).then_inc(s_exp)
    nc.scalar.wait_ge(s_in[len(CHUNKS) - 1], 16)
    nc.scalar.activation(E[:, TAIL_LO:TAIL_HI], x[:, TAIL_LO:TAIL_HI], AF.Exp)
    nc.scalar.activation(
        Lt[:], E[:, TAIL_LO:TAIL_HI], AF.Ln, bias=1.0, accum_out=out[:, 0:1]
    )
    nc.scalar.wait_ge(s_dve, 1)
    nc.scalar.activation(Lg[:], P3[:], AF.Ln, accum_out=out[:, 1:2]).then_inc(s_act)

    # DVE: per folded chunk, F = 1+E then three contiguous-halving products
    p3_off = 0
    for c, (lo, hi) in enumerate(FOLD_CHUNKS):
        w = hi - lo
        nc.vector.wait_ge(s_exp, c + 1)
        nc.vector.tensor_scalar_add(F[:, lo:hi], E[:, lo:hi], 1.0)
        h = w // 2
        nc.vector.tensor_mul(
            P1b[:, lo // 2 : lo // 2 + h], F[:, lo : lo + h], F[:, lo + h : hi]
        )
        q = h // 2
        p1 = P1b[:, lo // 2 : lo // 2 + h]
        nc.vector.tensor_mul(P2b[:, lo // 4 : lo // 4 + q], p1[:, :q], p1[:, q:])
        r = q // 2
        p2 = P2b[:, lo // 4 : lo // 4 + q]
        last = nc.vector.tensor_mul(P3[:, p3_off : p3_off + r], p2[:, :r], p2[:, r:])
        p3_off += r
    last.then_inc(s_dve)

    # SP: result out once both accumulator columns are written
    nc.sync.wait_ge(s_act, 1)
    nc.sync.dma_start(out=out_all[:], in_=out[:]).then_inc(s_out, 16)
    if WAIT_OUT:
        nc.sync.wait_ge(s_out, 16)

    # Restrict activation tables so exp and ln resolve to the combined
    # natural_log_exp_and_others set: one ACT table load for the whole
    # kernel.
    orig_get = bacc.get_activation_tables
    AFT = mybir.ActivationFunctionType

    def _combined_tables(arch):
        t = orig_get(arch)
        for name, fns in list(t.items()):
            if name != "natural_log_exp_and_others" and (
                AFT.Exp in fns or AFT.Ln in fns
            ):
                t[name] = {f for f in fns if f not in (AFT.Exp, AFT.Ln)}
        return t

    bacc.get_activation_tables = _combined_tables
    try:
        nc.finalize()
    finally:
        bacc.get_activation_tables = orig_get
    return nc


def get_program():
    if "nc" not in _CACHE:
        _CACHE["nc"] = _build_program()
    return _CACHE["nc"]


def _softplus(v):
    v = np.asarray(v, np.float32)
    return np.log1p(np.exp(-np.abs(v))) + np.maximum(v, np.float32(0.0))


def _host_positives(cls_logits, obj_logits, box_preds, gt_boxes, gt_labels, gt_masks):
    """First-come-wins assignment + all per-positive-cell loss terms, on host
    (float32, matching the reference's index math bit-exactly)."""
    B, N = gt_labels.shape
    gb = np.asarray(gt_boxes, dtype=np.float32)
    x = gb[..., 0]
    y = gb[..., 1]
    in_b = (x >= X_MIN) & (x <= X_MAX) & (y >= Y_MIN) & (y <= Y_MAX)
    gx = np.clip(np.floor((x - X_MIN) / RES).astype(np.int32), 0, BEV_W - 1)
    gy = np.clip(np.floor((y - Y_MIN) / RES).astype(np.int32), 0, BEV_H - 1)
    idx = gy * BEV_W + gx  # [B, N]
    valid = (
        (np.asarray(gt_masks, dtype=np.float32) > 0.5)
        & (np.asarray(gt_labels) >= 0)
        & in_b
    )

    f32 = np.float32
    s_neg = f32(0.0)
    s_pos = f32(0.0)
    s_ce = f32(0.0)
    s_box = f32(0.0)
    total_pos = 0
    for b in range(B):
        seen = set()
        for n in range(N):
            if not valid[b, n]:
                continue
            cell = int(idx[b, n])
            if cell in seen:
                continue
            seen.add(cell)
            total_pos += 1
            o = f32(obj_logits[b, cell])
            s_neg += _softplus(-o)
            s_pos += _softplus(o)
            cl = np.asarray(cls_logits[b, cell], f32)
            m = f32(cl.max())
            lse = m + f32(np.log(np.sum(np.exp(cl - m), dtype=f32)))
            s_ce += lse - f32(cl[int(gt_labels[b, n])])
            d = np.asarray(box_preds[b, cell], f32) - gb[b, n]
            ad = np.abs(d)
            sl1 = np.where(ad < 1.0, f32(0.5) * d * d, ad - f32(0.5))
            s_box += np.sum(sl1, dtype=f32)
    return total_pos, s_neg, s_pos, s_ce, s_box


def make_in_maps(obj_logits):
    in_maps = []
    for b in range(N_CORES):
        buf = np.asarray(obj_logits[b], dtype=np.float32).reshape(P_DIM, COLS)
        in_maps.append({"in_obj": buf.astype(IN_NP_DT)})
    return in_maps


def combine(results, pos_terms):
    """Host-side final reduction (float32 throughout)."""
    f32 = np.float32
    total_pos, s_neg, s_pos, s_ce, s_box = pos_terms
    s_all = f32(0.0)
    for res in results:
        s_all += np.sum(res["out_all"][:, 0:2].astype(np.float32), dtype=np.float32)

    M = f32(N_CORES * NUM_CELLS)
    positive = f32(total_pos)
    negatives = M - positive
    pos_weight = np.maximum(f32(1.0), negatives / (positive + f32(1e-6)))

    obj_loss = (s_all + pos_weight * s_neg - s_pos) / M
    if total_pos > 0:
        cls_loss = s_ce / np.maximum(positive, f32(1.0))
        box_loss = s_box / np.maximum(positive * f32(D), f32(1.0))
    else:
        cls_loss = f32(0.0)
        box_loss = f32(0.0)
    total = obj_loss + CLS_WEIGHT * cls_loss + BOX_WEIGHT * box_loss
    return np.array([total, cls_loss, box_loss, obj_loss], dtype=np.float32)


def kernel(cls_logits, obj_logits, box_preds, gt_boxes, gt_labels, gt_masks):
    cls_logits = np.asarray(cls_logits)
    obj_logits = np.asarray(obj_logits)
    box_preds = np.asarray(box_preds)
    B = obj_logits.shape[0]
    assert B == N_CORES, f"expected batch {N_CORES}, got {B}"

    pos_terms = _host_positives(
        cls_logits, obj_logits, box_preds, gt_boxes, gt_labels, gt_masks
    )

    nc = get_program()
    in_maps = make_in_maps(obj_logits)
    res = run_bass_kernel_spmd(nc, in_maps, list(range(N_CORES))).results
    return combine(res, pos_terms)


# revision 14
# speedup vs baseline: 1.2007x; 1.2007x over previous
"""BEV detection loss on 8 Trainium2 NeuronCores.

Strategy (data-parallel over batch, one batch element per core):
  - The loss touches cls_logits / box_preds only at positive cells (<= 64
    per batch element, from the first-come-wins scatter assignment).  The
    host does the tiny assignment + per-positive-cell math (CE, smooth-L1,
    softplus at positive cells) in float32; only the full scan
    sum(softplus(obj_logits)) over 262144 cells per core runs on device.
  - Device algorithm per core, on a [128, 2048] layout:
      x (fp8 e4m3, quantized on host -- 2e-2 rel-err budget, measured
        impact 2e-4) --DMA--> SBUF in 4 chunks on the SP HWDGE queue
      ACT: E = exp(x)            (bf16)
      DVE: F = 1 + E             (bf16, tensor-scalar 4x mode)
           three contiguous-halving products -> P = prod of 8 F's
           (group order is irrelevant for the sum; contiguous halves
           keep the DVE 2x packed mode)
      ACT: ln(P) over 256 cols with accum_out -> per-partition sums
    sum softplus = sum ln(1+e^x) = sum ln(P).  Products of 8 factors
    stay < e^44, well inside bf16 range.
  - Only the SP dynamic-HWDGE DMA queue is declared (dropping the unused
    Pool/Activation queues shortens the NEFF prelude/drain measurably).
  - Host combines per-core partial sums with the positives terms into the
    globally-consistent pos_weight and means (float32 throughout).
"""

import sys

import ml_dtypes
import numpy as np

sys.path.insert(0, "/opt/trn_rl_repo")

import concourse.bacc as bacc  # noqa: E402
import concourse.mybir as mybir  # noqa: E402
from concourse.bass_utils import run_bass_kernel_spmd  # noqa: E402

# BEV grid constants (must match the reference)
X_MIN = np.float32(-51.2)
X_MAX = np.float32(51.2)
Y_MIN = np.float32(-51.2)
Y_MAX = np.float32(51.2)
RES = np.float32(0.2)
BEV_W = 512
BEV_H = 512
NUM_CELLS = BEV_W * BEV_H  # 262144
CLS_WEIGHT = np.float32(1.0)
BOX_WEIGHT = np.float32(1.0)

N_CORES = 8
P_DIM = 128
COLS = NUM_CELLS // P_DIM  # 2048
C = 10
D = 7

import os

if os.environ.get("BEV_IN_DT", "fp8") == "bf16":
    IN_DT = mybir.dt.bfloat16
    IN_NP_DT = ml_dtypes.bfloat16
else:
    IN_DT = mybir.dt.float8e4
    IN_NP_DT = ml_dtypes.float8_e4m3
# chunk boundaries: small first chunk so ACT starts early; small last chunk
# (handled without DVE folding) so the post-last-arrival tail is short
FOLD_CHUNKS = [(0, 384), (384, 1088), (1088, 1792)]  # folded 8:1 via DVE products
TAIL_LO, TAIL_HI = 1792, 2048  # pure-ACT ln(1+e^x) chunk
CHUNKS = FOLD_CHUNKS + [(TAIL_LO, TAIL_HI)]
P3_COLS = sum((hi - lo) // 8 for lo, hi in FOLD_CHUNKS)  # 224
OUT_W = 6  # >=24B DMA descriptors (4B-descriptor DMAs complete slowly)
WAIT_OUT = os.environ.get("BEV_WAIT_OUT", "1") == "1"
PE_WARM = os.environ.get("BEV_PE_WARM", "0") == "1"


class _Bacc(bacc.Bacc):
    """Bacc whose constructor-time all_engine_barrier is skipped.

    The barrier only guards the const-pool memsets against engine use; this
    kernel first reads a const (the Ln bias) several microseconds after the
    memsets complete, so the barrier only delays the first input DMA."""

    def all_engine_barrier(self, **kw):
        return None

_CACHE = {}


def _build_program():
    f32 = mybir.dt.float32
    bf16 = mybir.dt.bfloat16
    AF = mybir.ActivationFunctionType

    nc = _Bacc("TRN2", debug=False, target_bir_lowering=False, num_devices=N_CORES)
    # Only the SP HWDGE dynamic queue is used; dropping the Pool/Act queue
    # declarations shortens NRT queue setup and the end-of-NEFF drain.
    nc.m.queues = [q for q in nc.m.queues if q.name == "qSPDynamicHW"]

    in_obj = nc.dram_tensor("in_obj", [P_DIM, COLS], IN_DT, kind="ExternalInput").ap()
    out_all = nc.dram_tensor("out_all", [P_DIM, OUT_W], f32, kind="ExternalOutput").ap()

    # Raw bass (no TileContext): dependencies are hand-managed semaphores.
    # Intra-engine ordering is program order — no semaphore chaining between
    # same-engine ops, which removes the per-op sync overhead and the tile
    # exit-block drains/barriers.
    x = nc.alloc_sbuf_tensor("x", [P_DIM, COLS], IN_DT)
    E = nc.alloc_sbuf_tensor("E", [P_DIM, COLS], bf16)
    F = nc.alloc_sbuf_tensor("F", [P_DIM, FOLD_CHUNKS[-1][1]], bf16)
    P1b = nc.alloc_sbuf_tensor("P1b", [P_DIM, FOLD_CHUNKS[-1][1] // 2], bf16)
    P2b = nc.alloc_sbuf_tensor("P2b", [P_DIM, FOLD_CHUNKS[-1][1] // 4], bf16)
    P3 = nc.alloc_sbuf_tensor("P3", [P_DIM, P3_COLS], bf16)
    Lt = nc.alloc_sbuf_tensor("Lt", [P_DIM, TAIL_HI - TAIL_LO], f32)
    Lg = nc.alloc_sbuf_tensor("Lg", [P_DIM, P3_COLS], f32)
    out = nc.alloc_sbuf_tensor("out", [P_DIM, OUT_W], f32)

    s_in = [nc.alloc_semaphore(f"s_in{c}") for c in range(len(CHUNKS))]
    s_exp = nc.alloc_semaphore("s_exp")
    s_dve = nc.alloc_semaphore("s_dve")
    s_act = nc.alloc_semaphore("s_act")
    s_out = nc.alloc_semaphore("s_out")

    # SP: input chunk DMAs, then the output DMA once ACT is done
    for c, (lo, hi) in enumerate(CHUNKS):
        nc.sync.dma_start(out=x[:, lo:hi], in_=in_obj[:, lo:hi]).then_inc(s_in[c], 16)

    if PE_WARM:
        # Dependency-free PE activity at block start: ramps the PE clock out
        # of its cold p-state so the PE sequencer processes the end-of-NEFF
        # semaphore-reset sequence faster. Reads an uninitialized scratch
        # tensor; the PSUM result is never used.
        dummy = nc.alloc_sbuf_tensor("pe_dummy", [P_DIM, 256], bf16)
        ps = nc.alloc_psum_tensor("pe_ps", [P_DIM, 256], f32)
        for _ in range(4):
            nc.tensor.matmul(
                ps[:], dummy[:, 0:128], dummy[:, 0:256], start=True, stop=True
            )

    # ACT: warmup (data-independent, pulls the exp/ln table load to block
    # start where it overlaps the input DMA), per-chunk exp, tail ln(1+e),
    # global ln over folded products
    nc.scalar.activation(Lg[:, 0:1], Lg[:, 0:1], AF.Exp, scale=0.0)
    for c, (lo, hi) in enumerate(FOLD_CHUNKS):
        nc.scalar.wait_ge(s_in[c], 16)
        nc.scalar.activation(E[:, lo:hi], x[:, lo:hi], AF.Exp).then_inc(s_exp)
    nc.scalar.wait_ge(s_in[len(CHUNKS) - 1], 16)
    nc.scalar.activation(E[:, TAIL_LO:TAIL_HI], x[:, TAIL_LO:TAIL_HI], AF.Exp)
    nc.scalar.activation(
        Lt[:], E[:, TAIL_LO:TAIL_HI], AF.Ln, bias=1.0, accum_out=out[:, 0:1]
    )
    nc.scalar.wait_ge(s_dve, 1)
    nc.scalar.activation(Lg[:], P3[:], AF.Ln, accum_out=out[:, 1:2]).then_inc(s_act)

    # DVE: per folded chunk, F = 1+E then three contiguous-halving products
    p3_off = 0
    for c, (lo, hi) in enumerate(FOLD_CHUNKS):
        w = hi - lo
        nc.vector.wait_ge(s_exp, c + 1)
        nc.vector.tensor_scalar_add(F[:, lo:hi], E[:, lo:hi], 1.0)
        h = w // 2
        nc.vector.tensor_mul(
            P1b[:, lo // 2 : lo // 2 + h], F[:, lo : lo + h], F[:, lo + h : hi]
        )
        q = h // 2
        p1 = P1b[:, lo // 2 : lo // 2 + h]
        nc.vector.tensor_mul(P2b[:, lo // 4 : lo // 4 + q], p1[:, :q], p1[:, q:])
        r = q // 2
        p2 = P2b[:, lo // 4 : lo // 4 + q]
        last = nc.vector.tensor_mul(P3[:, p3_off : p3_off + r], p2[:, :r], p2[:, r:])
        p3_off += r
    last.then_inc(s_dve)

    # SP: result out once both accumulator columns are written
    nc.sync.wait_ge(s_act, 1)
    nc.sync.dma_start(out=out_all[:], in_=out[:]).then_inc(s_out, 16)
    if WAIT_OUT:
        nc.sync.wait_ge(s_out, 16)

    # Restrict activation tables so exp and ln resolve to the combined
    # natural_log_exp_and_others set: one ACT table load for the whole
    # kernel.
    orig_get = bacc.get_activation_tables
    AFT = mybir.ActivationFunctionType

    def _combined_tables(arch):
        t = orig_get(arch)
        for name, fns in list(t.items()):
            if name != "natural_log_exp_and_others" and (
                AFT.Exp in fns or AFT.Ln in fns
            ):
                t[name] = {f for f in fns if f not in (AFT.Exp, AFT.Ln)}
        return t

    bacc.get_activation_tables = _combined_tables
    try:
        nc.finalize()
    finally:
        bacc.get_activation_tables = orig_get
    return nc


def get_program():
    if "nc" not in _CACHE:
        _CACHE["nc"] = _build_program()
    return _CACHE["nc"]


def _softplus(v):
    v = np.asarray(v, np.float32)
    return np.log1p(np.exp(-np.abs(v))) + np.maximum(v, np.float32(0.0))


def _host_positives(cls_logits, obj_logits, box_preds, gt_boxes, gt_labels, gt_masks):
    """First-come-wins assignment + all per-positive-cell loss terms, on host
    (float32, matching the reference's index math bit-exactly)."""
    B, N = gt_labels.shape
    gb = np.asarray(gt_boxes, dtype=np.float32)
    x = gb[..., 0]
    y = gb[..., 1]
    in_b = (x >= X_MIN) & (x <= X_MAX) & (y >= Y_MIN) & (y <= Y_MAX)
    gx = np.clip(np.floor((x - X_MIN) / RES).astype(np.int32), 0, BEV_W - 1)
    gy = np.clip(np.floor((y - Y_MIN) / RES).astype(np.int32), 0, BEV_H - 1)
    idx = gy * BEV_W + gx  # [B, N]
    valid = (
        (np.asarray(gt_masks, dtype=np.float32) > 0.5)
        & (np.asarray(gt_labels) >= 0)
        & in_b
    )

    f32 = np.float32
    s_neg = f32(0.0)
    s_pos = f32(0.0)
    s_ce = f32(0.0)
    s_box = f32(0.0)
    total_pos = 0
    for b in range(B):
        seen = set()
        for n in range(N):
            if not valid[b, n]:
                continue
            cell = int(idx[b, n])
            if cell in seen:
                continue
            seen.add(cell)
            total_pos += 1
            o = f32(obj_logits[b, cell])
            s_neg += _softplus(-o)
            s_pos += _softplus(o)
            cl = np.asarray(cls_logits[b, cell], f32)
            m = f32(cl.max())
            lse = m + f32(np.log(np.sum(np.exp(cl - m), dtype=f32)))
            s_ce += lse - f32(cl[int(gt_labels[b, n])])
            d = np.asarray(box_preds[b, cell], f32) - gb[b, n]
            ad = np.abs(d)
            sl1 = np.where(ad < 1.0, f32(0.5) * d * d, ad - f32(0.5))
            s_box += np.sum(sl1, dtype=f32)
    return total_pos, s_neg, s_pos, s_ce, s_box


def make_in_maps(obj_logits):
    in_maps = []
    for b in range(N_CORES):
        buf = np.asarray(obj_logits[b], dtype=np.float32).reshape(P_DIM, COLS)
        in_maps.append({"in_obj": buf.astype(IN_NP_DT)})
    return in_maps


def combine(results, pos_terms):
    """Host-side final reduction (float32 throughout)."""
    f32 = np.float32
    total_pos, s_neg, s_pos, s_ce, s_box = pos_terms
    s_all = f32(0.0)
    for res in results:
        s_all += np.sum(res["out_all"][:, 0:2].astype(np.float32), dtype=np.float32)

    M = f32(N_CORES * NUM_CELLS)
    positive = f32(total_pos)
    negatives = M - positive
    pos_weight = np.maximum(f32(1.0), negatives / (positive + f32(1e-6)))

    obj_loss = (s_all + pos_weight * s_neg - s_pos) / M
    if total_pos > 0:
        cls_loss = s_ce / np.maximum(positive, f32(1.0))
        box_loss = s_box / np.maximum(positive * f32(D), f32(1.0))
    else:
        cls_loss = f32(0.0)
        box_loss = f32(0.0)
    total = obj_loss + CLS_WEIGHT * cls_loss + BOX_WEIGHT * box_loss
    return np.array([total, cls_loss, box_loss, obj_loss], dtype=np.float32)


def kernel(cls_logits, obj_logits, box_preds, gt_boxes, gt_labels, gt_masks):
    cls_logits = np.asarray(cls_logits)
    obj_logits = np.asarray(obj_logits)
    box_preds = np.asarray(box_preds)
    B = obj_logits.shape[0]
    assert B == N_CORES, f"expected batch {N_CORES}, got {B}"

    pos_terms = _host_positives(
        cls_logits, obj_logits, box_preds, gt_boxes, gt_labels, gt_masks
    )

    nc = get_program()
    in_maps = make_in_maps(obj_logits)
    res = run_bass_kernel_spmd(nc, in_maps, list(range(N_CORES))).results
    return combine(res, pos_terms)
